# revision 1
# baseline (speedup 1.0000x reference)
"""2x2 average pool + per-channel affine on 8 TRN2 NeuronCores.

Problem: x (16, 64, 512, 512) f32 -> out (16, 64, 256, 256) f32
  out[b,c,i,j] = weight[c] * mean(x[b,c,2i:2i+2,2j:2j+2]) + bias[c]

Sharding: pure data parallel over batch. Core k gets batches [2k, 2k+1]
(128 images of 512x512 per core), weight/bias replicated.

Per-core layout: partition p = (b_local*64 + c) -> one full image per
partition. Each iteration DMAs 16 input rows per partition (32 KiB
contiguous, 4 MiB per dma_start), does the vertical pool with one
tensor_tensor add (row pairs are adjacent in the free dim), the
horizontal pool with a stride-2 tensor_tensor add, and the per-channel
affine on the scalar engine (scale/bias are per-partition [128,1]
scalars since partition == (b_local, c)).
"""

import numpy as np

import concourse.bacc as bacc
import concourse.bass as bass
import concourse.mybir as mybir
import concourse.tile as tile
from concourse.bass_utils import run_bass_kernel_spmd

N_CORES = 8
B, C, S = 16, 64, 512
B_LOC = B // N_CORES            # 2 batches per core
P = B_LOC * C                   # 128 partitions = one image per partition
IMG = S * S                     # 262144 input elems per image
OS = S // 2                     # 256
OUT_IMG = OS * OS               # 65536 output elems per image
ROWS_PER_ITER = 16              # input rows loaded per iteration
CHUNK = ROWS_PER_ITER * S       # 8192 elems per partition per load (32 KiB)
N_ITERS = IMG // CHUNK          # 32
OUT_CHUNK = CHUNK // 4          # 2048 elems per partition per store

FP32 = mybir.dt.float32

_nc_cache = None


def _build(reps=1, rows=8, ibufs=6, vbufs=3, hbufs=3, obufs=4,
           store_eng="scalar", split_load=1, loop_n=0,
           inplace_v=False, inplace_y=False):
    # Bacc (not raw Bass): its finalize pass splits multi-sem waits into
    # event-semaphore instructions — TRN2 allows at most 1 wait per inst.
    # reps>1 repeats the full pass back-to-back in one NEFF (delta-timing).
    nc = bacc.Bacc("TRN2", target_bir_lowering=False, debug=False,
                   num_devices=N_CORES)
    chunk = rows * S             # input elems per partition per iteration
    n_iters = IMG // chunk
    out_chunk = chunk // 4

    x = nc.declare_dram_parameter("x", [P, IMG], FP32, isOutput=False)
    # affine[:, 0] = weight[c] / 4 (pool norm folded in), affine[:, 1] = bias[c]
    # (host-precomputed, already broadcast to the 128 partition images)
    affine = nc.declare_dram_parameter("affine", [P, 2], FP32, isOutput=False)
    out = nc.declare_dram_parameter("out", [P, OUT_IMG], FP32, isOutput=True)

    store = {"sync": nc.sync, "scalar": nc.scalar, "gpsimd": nc.gpsimd}[store_eng]

    with tile.TileContext(nc) as tc:
        with tc.tile_pool(name="consts", bufs=1) as cpool, \
             tc.tile_pool(name="ld", bufs=ibufs) as ipool, \
             tc.tile_pool(name="vmid", bufs=vbufs) as vpool, \
             tc.tile_pool(name="hmid", bufs=hbufs) as hpool, \
             tc.tile_pool(name="st", bufs=obufs) as opool:

            cb = cpool.tile([P, 2], FP32)
            nc.sync.dma_start(out=cb[:], in_=affine[:, :])
            s_ap = cb[:, 0:1]
            b_ap = cb[:, 1:2]

            import contextlib
            loop_ctx = tc.For_i(0, loop_n, 1) if loop_n else \
                contextlib.nullcontext()
            with loop_ctx:
              for i in range(n_iters * reps):
                i = i % n_iters
                t = ipool.tile([P, chunk], FP32)
                if split_load == 1:
                    nc.sync.dma_start(out=t[:],
                                      in_=x[:, i * chunk:(i + 1) * chunk])
                else:
                    part = chunk // split_load
                    for s_ in range(split_load):
                        nc.sync.dma_start(
                            out=t[:, s_ * part:(s_ + 1) * part],
                            in_=x[:, i * chunk + s_ * part:
                                  i * chunk + (s_ + 1) * part])

                # vertical pool: rows 2r and 2r+1 sit at free-dim offsets
                # (2r*S, (2r+1)*S) -> contiguous-stride add. Writing the
                # result into the front of t is safe: the DVE streams
                # monotonically and every write index trails its reads.
                tv = t[:].rearrange("p (r two w) -> p r two w", two=2, w=S)
                v_ap = t[:, 0:chunk // 2] if inplace_v else \
                    vpool.tile([P, chunk // 2], FP32, name="v", tag="v")[:]
                vv = v_ap.rearrange("p (r w) -> p r w", w=S)
                nc.vector.tensor_add(vv, tv[:, :, 0, :], tv[:, :, 1, :])

                # horizontal pool: adjacent column pairs, stride-2 operands
                vh = v_ap.rearrange("p (r j two) -> p r j two", two=2, j=OS)
                y = opool.tile([P, out_chunk], FP32)
                h_ap = y[:] if inplace_y else \
                    hpool.tile([P, out_chunk], FP32, name="h", tag="h")[:]
                hh = h_ap.rearrange("p (r j) -> p r j", j=OS)
                nc.vector.tensor_add(hh, vh[:, :, :, 0], vh[:, :, :, 1])

                # per-channel affine on the scalar engine:
                # y = Identity(h * (w[c]/4) + bias[c])
                nc.scalar.activation(y[:], h_ap,
                                     mybir.ActivationFunctionType.Identity,
                                     bias=b_ap, scale=s_ap)

                store.dma_start(out=out[:, i * out_chunk:(i + 1) * out_chunk],
                                in_=y[:])

    # run Bacc's legalization passes (multi-wait splitting, reg alloc, ...);
    # run_bass_via_pjrt serializes nc.m as-is and never finalizes.
    nc.finalize()
    return nc


def _get_nc():
    global _nc_cache
    if _nc_cache is None:
        _nc_cache = _build()
    return _nc_cache


def _make_in_maps(x, weight, bias):
    x = np.ascontiguousarray(np.asarray(x, dtype=np.float32))
    weight = np.asarray(weight, dtype=np.float32).reshape(C)
    bias = np.asarray(bias, dtype=np.float32).reshape(C)
    affine = np.stack([np.tile(weight * 0.25, B_LOC),
                       np.tile(bias, B_LOC)], axis=1)
    affine = np.ascontiguousarray(affine, dtype=np.float32)  # [P, 2]
    in_maps = []
    for k in range(N_CORES):
        shard = np.ascontiguousarray(
            x[k * B_LOC:(k + 1) * B_LOC].reshape(P, IMG))
        in_maps.append({"x": shard, "affine": affine})
    return in_maps


def run_sharded(x, weight, bias, trace=False, build_kw=None, **kw):
    """Run the SPMD kernel; returns (full_output, BassKernelResults)."""
    nc = _build(**build_kw) if build_kw else _get_nc()
    res = run_bass_kernel_spmd(nc, _make_in_maps(x, weight, bias),
                               core_ids=list(range(N_CORES)), trace=trace, **kw)
    outs = [res.results[k]["out"].reshape(B_LOC, C, OS, OS)
            for k in range(N_CORES)]
    return np.concatenate(outs, axis=0), res


def kernel(x, weight, bias):
    out, _ = run_sharded(x, weight, bias, trace=False)
    return out



# revision 3
# speedup vs baseline: 1.0460x; 1.0460x over previous
"""2x2 average pool + per-channel affine on 8 TRN2 NeuronCores.

Problem: x (16, 64, 512, 512) f32 -> out (16, 64, 256, 256) f32
  out[b,c,i,j] = weight[c] * mean(x[b,c,2i:2i+2,2j:2j+2]) + bias[c]

Sharding: pure data parallel over batch. Core k gets batches [2k, 2k+1]
(128 images of 512x512 per core), weight/bias replicated.

Per-core layout: partition p = (b_local*64 + c) -> one full image per
partition. The kernel streams row-chunks: DMA `rows` input rows per
partition (contiguous in the free dim), vertical pool with one
tensor_tensor add (row pairs adjacent in the free dim, written in-place
into the front of the load tile), horizontal pool with a stride-2
tensor_tensor add into the output tile, per-channel affine on the
scalar engine (scale/bias are per-partition [128,1] scalars since
partition == (b_local, c)), store.

This is HBM-bandwidth-bound (167.8 MB per core at ~350 GB/s when all
8 cores contend). The schedule tapers the last chunks so the
compute+store drain after the final load is short, and the affine
constant loads on the gpsimd queue so the sync ring starts the first
big load immediately.
"""

import numpy as np

import concourse.bacc as bacc
import concourse.bass as bass
import concourse.mybir as mybir
import concourse.tile as tile
from concourse.bass_utils import run_bass_kernel_spmd

N_CORES = 8
B, C, S = 16, 64, 512
B_LOC = B // N_CORES            # 2 batches per core
P = B_LOC * C                   # 128 partitions = one image per partition
IMG = S * S                     # 262144 input elems per image
OS = S // 2                     # 256
OUT_IMG = OS * OS               # 65536 output elems per image

FP32 = mybir.dt.float32

_nc_cache = None


def _build(reps=1, rows=8, ibufs=6, vbufs=0, hbufs=0, obufs=4,
           store_eng="scalar", const_eng="gpsimd", split_load=1,
           inplace_v=True, inplace_y=True, taper=(4, 2, 2),
           loop_n=0):
    # Bacc (not raw Bass): its finalize pass splits multi-sem waits into
    # event-semaphore instructions — TRN2 allows at most 1 wait per inst.
    nc = bacc.Bacc("TRN2", target_bir_lowering=False, debug=False,
                   num_devices=N_CORES)

    # row schedule: n full chunks of `rows`, then the taper (drain
    # shortener — small chunks finish compute+store quickly after the
    # last big load lands)
    taper = tuple(r for r in taper if r)
    t_rows = sum(taper)
    assert t_rows % 2 == 0 and all(r % 2 == 0 for r in taper)
    n_full = (S - t_rows) // rows
    assert n_full * rows + t_rows == S
    sched = [rows] * n_full + list(taper)

    x = nc.declare_dram_parameter("x", [P, IMG], FP32, isOutput=False)
    # affine[:, 0] = weight[c] / 4 (pool norm folded in), affine[:, 1] = bias[c]
    # (host-precomputed, already broadcast to the 128 partition images)
    affine = nc.declare_dram_parameter("affine", [P, 2], FP32, isOutput=False)
    out = nc.declare_dram_parameter("out", [P, OUT_IMG], FP32, isOutput=True)

    engs = {"sync": nc.sync, "scalar": nc.scalar, "gpsimd": nc.gpsimd}
    store = engs[store_eng]
    const = engs[const_eng]

    max_chunk = rows * S

    with tile.TileContext(nc) as tc:
        with tc.tile_pool(name="consts", bufs=1) as cpool, \
             tc.tile_pool(name="ld", bufs=ibufs) as ipool, \
             tc.tile_pool(name="vmid", bufs=max(vbufs, 1)) as vpool, \
             tc.tile_pool(name="hmid", bufs=max(hbufs, 1)) as hpool, \
             tc.tile_pool(name="st", bufs=obufs) as opool:

            cb = cpool.tile([P, 2], FP32)
            const.dma_start(out=cb[:], in_=affine[:, :])
            s_ap = cb[:, 0:1]
            b_ap = cb[:, 1:2]

            import contextlib
            loop_ctx = tc.For_i(0, loop_n, 1) if loop_n else \
                contextlib.nullcontext()
            with loop_ctx:
              for rep in range(reps):
                off = 0          # input elem offset per partition
                ooff = 0         # output elem offset per partition
                for r_i in sched:
                    chunk = r_i * S
                    out_chunk = chunk // 4
                    t = ipool.tile([P, chunk], FP32, tag="ld")
                    if split_load == 1:
                        nc.sync.dma_start(out=t[:],
                                          in_=x[:, off:off + chunk])
                    else:
                        part = chunk // split_load
                        for s_ in range(split_load):
                            nc.sync.dma_start(
                                out=t[:, s_ * part:(s_ + 1) * part],
                                in_=x[:, off + s_ * part:
                                      off + (s_ + 1) * part])

                    # vertical pool: rows 2r and 2r+1 sit at free-dim
                    # offsets (2r*S, (2r+1)*S) -> contiguous-stride add.
                    # Writing the result into the front of t is safe: the
                    # DVE streams monotonically and every write index
                    # trails its reads.
                    tv = t[:].rearrange("p (r two w) -> p r two w",
                                        two=2, w=S)
                    v_ap = t[:, 0:chunk // 2] if inplace_v else \
                        vpool.tile([P, chunk // 2], FP32, name="v",
                                   tag="v")[:]
                    vv = v_ap.rearrange("p (r w) -> p r w", w=S)
                    nc.vector.tensor_add(vv, tv[:, :, 0, :], tv[:, :, 1, :])

                    # horizontal pool: adjacent column pairs, stride-2
                    vh = v_ap.rearrange("p (r j two) -> p r j two",
                                        two=2, j=OS)
                    y = opool.tile([P, out_chunk], FP32, tag="st")
                    h_ap = y[:] if inplace_y else \
                        hpool.tile([P, out_chunk], FP32, name="h",
                                   tag="h")[:]
                    hh = h_ap.rearrange("p (r j) -> p r j", j=OS)
                    nc.vector.tensor_add(hh, vh[:, :, :, 0], vh[:, :, :, 1])

                    # per-channel affine on the scalar engine:
                    # y = Identity(h * (w[c]/4) + bias[c])
                    nc.scalar.activation(y[:], h_ap,
                                         mybir.ActivationFunctionType.Identity,
                                         bias=b_ap, scale=s_ap)

                    store.dma_start(out=out[:, ooff:ooff + out_chunk],
                                    in_=y[:])
                    off += chunk
                    ooff += out_chunk

    # run Bacc's legalization passes (multi-wait splitting, reg alloc, ...);
    # run_bass_via_pjrt serializes nc.m as-is and never finalizes.
    nc.finalize()
    return nc


def _get_nc():
    global _nc_cache
    if _nc_cache is None:
        _nc_cache = _build()
    return _nc_cache


def _make_in_maps(x, weight, bias):
    x = np.ascontiguousarray(np.asarray(x, dtype=np.float32))
    weight = np.asarray(weight, dtype=np.float32).reshape(C)
    bias = np.asarray(bias, dtype=np.float32).reshape(C)
    affine = np.stack([np.tile(weight * 0.25, B_LOC),
                       np.tile(bias, B_LOC)], axis=1)
    affine = np.ascontiguousarray(affine, dtype=np.float32)  # [P, 2]
    in_maps = []
    for k in range(N_CORES):
        shard = np.ascontiguousarray(
            x[k * B_LOC:(k + 1) * B_LOC].reshape(P, IMG))
        in_maps.append({"x": shard, "affine": affine})
    return in_maps


def run_sharded(x, weight, bias, trace=False, build_kw=None, **kw):
    """Run the SPMD kernel; returns (full_output, BassKernelResults)."""
    nc = _build(**build_kw) if build_kw is not None else _get_nc()
    res = run_bass_kernel_spmd(nc, _make_in_maps(x, weight, bias),
                               core_ids=list(range(N_CORES)), trace=trace, **kw)
    outs = [res.results[k]["out"].reshape(B_LOC, C, OS, OS)
            for k in range(N_CORES)]
    return np.concatenate(outs, axis=0), res


def kernel(x, weight, bias):
    out, _ = run_sharded(x, weight, bias, trace=False)
    return out


# revision 5
# speedup vs baseline: 1.0493x; 1.0032x over previous
"""2x2 average pool + per-channel affine on 8 TRN2 NeuronCores.

Problem: x (16, 64, 512, 512) f32 -> out (16, 64, 256, 256) f32
  out[b,c,i,j] = weight[c] * mean(x[b,c,2i:2i+2,2j:2j+2]) + bias[c]

Sharding: pure data parallel over batch. Core k gets batches [2k, 2k+1]
(128 images of 512x512 per core), weight/bias replicated.

Per-core layout: partition p = (b_local*64 + c) -> one full image per
partition. The kernel streams row-chunks: DMA `rows` input rows per
partition (contiguous in the free dim) on the sync HWDGE ring,
vertical pool with one tensor_tensor add (row pairs adjacent in the
free dim), horizontal pool with a stride-2 tensor_tensor add,
per-channel affine on the scalar engine (scale/bias are per-partition
[128,1] scalars since partition == (b_local, c)), store on the scalar
HWDGE ring.

This is DMA-bandwidth-bound: 167.8 MB of HBM traffic per core, ~430
GB/s per-core ceiling when the chip is quiet (measured), less under
cross-core/tenant contention. Two scheduling details claw back the
head/tail overhead: the row schedule tapers the last chunks (4,2,2)
so the compute+store drain after the final load is short, and the
affine constant loads on the gpsimd (SWDGE) queue so the sync ring
issues the first big load immediately at t=0.
"""

import numpy as np

import concourse.bacc as bacc
import concourse.bass as bass
import concourse.mybir as mybir
import concourse.tile as tile
from concourse.bass_utils import run_bass_kernel_spmd

N_CORES = 8
B, C, S = 16, 64, 512
B_LOC = B // N_CORES            # 2 batches per core
P = B_LOC * C                   # 128 partitions = one image per partition
IMG = S * S                     # 262144 input elems per image
OS = S // 2                     # 256
OUT_IMG = OS * OS               # 65536 output elems per image

FP32 = mybir.dt.float32

_nc_cache = None


def _build(reps=1, rows=8, ibufs=6, vbufs=3, hbufs=3, obufs=4,
           store_eng="scalar", const_eng="gpsimd", split_load=1,
           inplace_v=False, inplace_y=False, taper=(4, 2, 2),
           loop_n=0):
    # Bacc (not raw Bass): its finalize pass splits multi-sem waits into
    # event-semaphore instructions — TRN2 allows at most 1 wait per inst.
    nc = bacc.Bacc("TRN2", target_bir_lowering=False, debug=False,
                   num_devices=N_CORES)

    # row schedule: n full chunks of `rows`, then the taper (drain
    # shortener — small chunks finish compute+store quickly after the
    # last big load lands)
    taper = tuple(r for r in taper if r)
    t_rows = sum(taper)
    assert t_rows % 2 == 0 and all(r % 2 == 0 for r in taper)
    n_full = (S - t_rows) // rows
    assert n_full * rows + t_rows == S
    sched = [rows] * n_full + list(taper)

    x = nc.declare_dram_parameter("x", [P, IMG], FP32, isOutput=False)
    # affine[:, 0] = weight[c] / 4 (pool norm folded in), affine[:, 1] = bias[c]
    # (host-precomputed, already broadcast to the 128 partition images)
    affine = nc.declare_dram_parameter("affine", [P, 2], FP32, isOutput=False)
    out = nc.declare_dram_parameter("out", [P, OUT_IMG], FP32, isOutput=True)

    engs = {"sync": nc.sync, "scalar": nc.scalar, "gpsimd": nc.gpsimd}
    store = engs[store_eng]
    const = engs[const_eng]

    max_chunk = rows * S

    with tile.TileContext(nc) as tc:
        with tc.tile_pool(name="consts", bufs=1) as cpool, \
             tc.tile_pool(name="ld", bufs=ibufs) as ipool, \
             tc.tile_pool(name="vmid", bufs=max(vbufs, 1)) as vpool, \
             tc.tile_pool(name="hmid", bufs=max(hbufs, 1)) as hpool, \
             tc.tile_pool(name="st", bufs=obufs) as opool:

            cb = cpool.tile([P, 2], FP32)
            const.dma_start(out=cb[:], in_=affine[:, :])
            s_ap = cb[:, 0:1]
            b_ap = cb[:, 1:2]

            import contextlib
            loop_ctx = tc.For_i(0, loop_n, 1) if loop_n else \
                contextlib.nullcontext()
            with loop_ctx:
              for rep in range(reps):
                off = 0          # input elem offset per partition
                ooff = 0         # output elem offset per partition
                for r_i in sched:
                    chunk = r_i * S
                    out_chunk = chunk // 4
                    t = ipool.tile([P, chunk], FP32, tag="ld")
                    if split_load == 1:
                        nc.sync.dma_start(out=t[:],
                                          in_=x[:, off:off + chunk])
                    else:
                        part = chunk // split_load
                        for s_ in range(split_load):
                            nc.sync.dma_start(
                                out=t[:, s_ * part:(s_ + 1) * part],
                                in_=x[:, off + s_ * part:
                                      off + (s_ + 1) * part])

                    # vertical pool: rows 2r and 2r+1 sit at free-dim
                    # offsets (2r*S, (2r+1)*S) -> contiguous-stride add.
                    # Writing the result into the front of t is safe: the
                    # DVE streams monotonically and every write index
                    # trails its reads.
                    tv = t[:].rearrange("p (r two w) -> p r two w",
                                        two=2, w=S)
                    v_ap = t[:, 0:chunk // 2] if inplace_v else \
                        vpool.tile([P, chunk // 2], FP32, name="v",
                                   tag="v")[:]
                    vv = v_ap.rearrange("p (r w) -> p r w", w=S)
                    nc.vector.tensor_add(vv, tv[:, :, 0, :], tv[:, :, 1, :])

                    # horizontal pool: adjacent column pairs, stride-2
                    vh = v_ap.rearrange("p (r j two) -> p r j two",
                                        two=2, j=OS)
                    y = opool.tile([P, out_chunk], FP32, tag="st")
                    h_ap = y[:] if inplace_y else \
                        hpool.tile([P, out_chunk], FP32, name="h",
                                   tag="h")[:]
                    hh = h_ap.rearrange("p (r j) -> p r j", j=OS)
                    nc.vector.tensor_add(hh, vh[:, :, :, 0], vh[:, :, :, 1])

                    # per-channel affine on the scalar engine:
                    # y = Identity(h * (w[c]/4) + bias[c])
                    nc.scalar.activation(y[:], h_ap,
                                         mybir.ActivationFunctionType.Identity,
                                         bias=b_ap, scale=s_ap)

                    store.dma_start(out=out[:, ooff:ooff + out_chunk],
                                    in_=y[:])
                    off += chunk
                    ooff += out_chunk

    # run Bacc's legalization passes (multi-wait splitting, reg alloc, ...);
    # run_bass_via_pjrt serializes nc.m as-is and never finalizes.
    nc.finalize()
    return nc


def _get_nc():
    global _nc_cache
    if _nc_cache is None:
        _nc_cache = _build()
    return _nc_cache


def _make_in_maps(x, weight, bias):
    x = np.ascontiguousarray(np.asarray(x, dtype=np.float32))
    weight = np.asarray(weight, dtype=np.float32).reshape(C)
    bias = np.asarray(bias, dtype=np.float32).reshape(C)
    affine = np.stack([np.tile(weight * 0.25, B_LOC),
                       np.tile(bias, B_LOC)], axis=1)
    affine = np.ascontiguousarray(affine, dtype=np.float32)  # [P, 2]
    in_maps = []
    for k in range(N_CORES):
        shard = np.ascontiguousarray(
            x[k * B_LOC:(k + 1) * B_LOC].reshape(P, IMG))
        in_maps.append({"x": shard, "affine": affine})
    return in_maps


def run_sharded(x, weight, bias, trace=False, build_kw=None, **kw):
    """Run the SPMD kernel; returns (full_output, BassKernelResults)."""
    nc = _build(**build_kw) if build_kw is not None else _get_nc()
    res = run_bass_kernel_spmd(nc, _make_in_maps(x, weight, bias),
                               core_ids=list(range(N_CORES)), trace=trace, **kw)
    outs = [res.results[k]["out"].reshape(B_LOC, C, OS, OS)
            for k in range(N_CORES)]
    return np.concatenate(outs, axis=0), res


def kernel(x, weight, bias):
    out, _ = run_sharded(x, weight, bias, trace=False)
    return out


# revision 7
# speedup vs baseline: 1.0731x; 1.0226x over previous
"""2x2 average pool + per-channel affine on 8 TRN2 NeuronCores.

Problem: x (16, 64, 512, 512) f32 -> out (16, 64, 256, 256) f32
  out[b,c,i,j] = weight[c] * mean(x[b,c,2i:2i+2,2j:2j+2]) + bias[c]

Sharding: pure data parallel over batch. Core k gets batches [2k, 2k+1]
(128 images of 512x512 per core), weight/bias replicated.

Per-core layout: partition p = (b_local*64 + c) -> one full image per
partition. The kernel streams row-chunks: DMA `rows` input rows per
partition (contiguous in the free dim) on the sync HWDGE ring,
vertical pool with one tensor_tensor add (row pairs adjacent in the
free dim), horizontal pool with a stride-2 tensor_tensor add,
per-channel affine on the scalar engine (scale/bias are per-partition
[128,1] scalars since partition == (b_local, c)), store on the scalar
HWDGE ring.

This is DMA-bandwidth-bound: 167.8 MB of HBM traffic per core, ~430
GB/s per-core ceiling when the chip is quiet (measured), less under
cross-core/tenant contention. Scheduling details that claw back the
head/tail overhead: 16-row chunks (4 MiB loads) keep the unrolled
iteration count at 34, halving the ~9 us serial EVENT_SEMAPHORE
teardown the Bacc postamble runs per cross-engine event; the row
schedule tapers the last chunks (8,4,2,2) so the compute+store drain
after the final load is short; and the affine constant loads on the
scalar HWDGE ring (not gpsimd — SWDGE init adds descriptor-ring
MEMSETs and a long GpSimd drain to the preamble) so the sync ring
issues the first big load immediately.
"""

import numpy as np

import concourse.bacc as bacc
import concourse.bass as bass
import concourse.mybir as mybir
import concourse.tile as tile
from concourse.bass_utils import run_bass_kernel_spmd

N_CORES = 8
B, C, S = 16, 64, 512
B_LOC = B // N_CORES            # 2 batches per core
P = B_LOC * C                   # 128 partitions = one image per partition
IMG = S * S                     # 262144 input elems per image
OS = S // 2                     # 256
OUT_IMG = OS * OS               # 65536 output elems per image

FP32 = mybir.dt.float32

_nc_cache = None


def _build(reps=1, rows=16, ibufs=4, vbufs=2, hbufs=2, obufs=3,
           store_eng="scalar", const_eng="scalar", split_load=1,
           inplace_v=False, inplace_y=False, taper=(8, 4, 2, 2),
           loop_n=0):
    # Bacc (not raw Bass): its finalize pass splits multi-sem waits into
    # event-semaphore instructions — TRN2 allows at most 1 wait per inst.
    nc = bacc.Bacc("TRN2", target_bir_lowering=False, debug=False,
                   num_devices=N_CORES)

    # row schedule: n full chunks of `rows`, then the taper (drain
    # shortener — small chunks finish compute+store quickly after the
    # last big load lands)
    taper = tuple(r for r in taper if r)
    t_rows = sum(taper)
    assert t_rows % 2 == 0 and all(r % 2 == 0 for r in taper)
    n_full = (S - t_rows) // rows
    assert n_full * rows + t_rows == S
    sched = [rows] * n_full + list(taper)

    x = nc.declare_dram_parameter("x", [P, IMG], FP32, isOutput=False)
    # affine[:, 0] = weight[c] / 4 (pool norm folded in), affine[:, 1] = bias[c]
    # (host-precomputed, already broadcast to the 128 partition images)
    affine = nc.declare_dram_parameter("affine", [P, 2], FP32, isOutput=False)
    out = nc.declare_dram_parameter("out", [P, OUT_IMG], FP32, isOutput=True)

    engs = {"sync": nc.sync, "scalar": nc.scalar, "gpsimd": nc.gpsimd}
    store = engs[store_eng]
    const = engs[const_eng]

    max_chunk = rows * S

    with tile.TileContext(nc) as tc:
        with tc.tile_pool(name="consts", bufs=1) as cpool, \
             tc.tile_pool(name="ld", bufs=ibufs) as ipool, \
             tc.tile_pool(name="vmid", bufs=max(vbufs, 1)) as vpool, \
             tc.tile_pool(name="hmid", bufs=max(hbufs, 1)) as hpool, \
             tc.tile_pool(name="st", bufs=obufs) as opool:

            cb = cpool.tile([P, 2], FP32)
            const.dma_start(out=cb[:], in_=affine[:, :])
            s_ap = cb[:, 0:1]
            b_ap = cb[:, 1:2]

            import contextlib
            loop_ctx = tc.For_i(0, loop_n, 1) if loop_n else \
                contextlib.nullcontext()
            with loop_ctx:
              for rep in range(reps):
                off = 0          # input elem offset per partition
                ooff = 0         # output elem offset per partition
                for r_i in sched:
                    chunk = r_i * S
                    out_chunk = chunk // 4
                    t = ipool.tile([P, chunk], FP32, tag="ld")
                    if split_load == 1:
                        nc.sync.dma_start(out=t[:],
                                          in_=x[:, off:off + chunk])
                    else:
                        part = chunk // split_load
                        for s_ in range(split_load):
                            nc.sync.dma_start(
                                out=t[:, s_ * part:(s_ + 1) * part],
                                in_=x[:, off + s_ * part:
                                      off + (s_ + 1) * part])

                    # vertical pool: rows 2r and 2r+1 sit at free-dim
                    # offsets (2r*S, (2r+1)*S) -> contiguous-stride add.
                    # Writing the result into the front of t is safe: the
                    # DVE streams monotonically and every write index
                    # trails its reads.
                    tv = t[:].rearrange("p (r two w) -> p r two w",
                                        two=2, w=S)
                    v_ap = t[:, 0:chunk // 2] if inplace_v else \
                        vpool.tile([P, chunk // 2], FP32, name="v",
                                   tag="v")[:]
                    vv = v_ap.rearrange("p (r w) -> p r w", w=S)
                    nc.vector.tensor_add(vv, tv[:, :, 0, :], tv[:, :, 1, :])

                    # horizontal pool: adjacent column pairs, stride-2
                    vh = v_ap.rearrange("p (r j two) -> p r j two",
                                        two=2, j=OS)
                    y = opool.tile([P, out_chunk], FP32, tag="st")
                    h_ap = y[:] if inplace_y else \
                        hpool.tile([P, out_chunk], FP32, name="h",
                                   tag="h")[:]
                    hh = h_ap.rearrange("p (r j) -> p r j", j=OS)
                    nc.vector.tensor_add(hh, vh[:, :, :, 0], vh[:, :, :, 1])

                    # per-channel affine on the scalar engine:
                    # y = Identity(h * (w[c]/4) + bias[c])
                    nc.scalar.activation(y[:], h_ap,
                                         mybir.ActivationFunctionType.Identity,
                                         bias=b_ap, scale=s_ap)

                    store.dma_start(out=out[:, ooff:ooff + out_chunk],
                                    in_=y[:])
                    off += chunk
                    ooff += out_chunk

    # run Bacc's legalization passes (multi-wait splitting, reg alloc, ...);
    # run_bass_via_pjrt serializes nc.m as-is and never finalizes.
    nc.finalize()
    return nc


def _get_nc():
    global _nc_cache
    if _nc_cache is None:
        _nc_cache = _build()
    return _nc_cache


def _make_in_maps(x, weight, bias):
    x = np.ascontiguousarray(np.asarray(x, dtype=np.float32))
    weight = np.asarray(weight, dtype=np.float32).reshape(C)
    bias = np.asarray(bias, dtype=np.float32).reshape(C)
    affine = np.stack([np.tile(weight * 0.25, B_LOC),
                       np.tile(bias, B_LOC)], axis=1)
    affine = np.ascontiguousarray(affine, dtype=np.float32)  # [P, 2]
    in_maps = []
    for k in range(N_CORES):
        shard = np.ascontiguousarray(
            x[k * B_LOC:(k + 1) * B_LOC].reshape(P, IMG))
        in_maps.append({"x": shard, "affine": affine})
    return in_maps


def run_sharded(x, weight, bias, trace=False, build_kw=None, **kw):
    """Run the SPMD kernel; returns (full_output, BassKernelResults)."""
    nc = _build(**build_kw) if build_kw is not None else _get_nc()
    res = run_bass_kernel_spmd(nc, _make_in_maps(x, weight, bias),
                               core_ids=list(range(N_CORES)), trace=trace, **kw)
    outs = [res.results[k]["out"].reshape(B_LOC, C, OS, OS)
            for k in range(N_CORES)]
    return np.concatenate(outs, axis=0), res


def kernel(x, weight, bias):
    out, _ = run_sharded(x, weight, bias, trace=False)
    return out
